# revision 31
# baseline (speedup 1.0000x reference)
"""Trainium2 Bass kernel for nn_AttentionLayer (B=64, S=512, F=256), 8 cores.

Reference computation (per batch b):
    scores = x1 @ Wq + x2 @ Wk          # [S, S]
    a = softmax(tanh(scores), axis=-1)   # softmax over u
    a2 = a @ Wv                          # [S, S]
    out = a2 * x1                        # elementwise
    out = out * rsqrt(max(sum_s out^2, eps))   # l2-normalize over axis s

Strategy: pure data parallelism -- 8 batches per core, weights replicated.
Everything is computed in a TRANSPOSED layout ([t-or-u partitions, s free]).

Final design notes (informed by HW traces):
  * x1 is DMA'd ONCE as float32r: stage A streams it as the moving matmul
    operand (1 cycle/row -- same PE rate as bf16), and the epilogue
    bitcasts the same SBUF bytes to f32.  A second bf16 copy of x1 was
    measured to stretch every engine ~20% via SBUF port contention.
  * All weights + x2 are bf16 (halves their DMA; bf16 stationary loads
    avoid the f32r weight-load stretch on stage-A instructions).
  * fp8 DoubleRow measures only 2x per MAC on this HW, so accuracy-safe
    hi+lo digit splits lose to f32r/bf16 -- not used.
  * Stage C consumes the UNNORMALIZED expz; 1/denominator folds into the
    epilogue, so no PE work waits on the rowsum->recip->broadcast chain.
  * Epilogue per t-tile: q = y*x1 and w = q*recip_bc as dtype-pure f32
    tensor_tensor on DVE (mixed-dtype DVE ops hit a ~2.6x slow path);
    sum-of-squares split between ACT Square+accum (Square lives in both
    activation-table sets -- never forces a swap) and DVE stt; final
    per-row 1/sqrt scale on GpSimd normalize_recip (native Q7 op; gpsimd
    tensor_scalar is a ~7.6us/tile software trap -- never use).
  * The softmax rowsum uses a ones-BLOCK stationary so the matmul writes
    the denominator replicated across all 128 partitions -- the reciprocal
    lands directly in broadcast form and GpSimd partition_broadcast is
    eliminated.
  * Dummy matmuls keep the TensorE p-state high through the startup DMA
    window and the drain's exp-wait (the device otherwise re-ramps from
    1.2GHz, stretching instructions 427 vs 216ns).
  * Sqrt on ACT for batch PAIRS; the drain shares its sqrt-table epoch.
  * Drain finalize: normalizes split Pool/DVE, output in two half-DMAs.
  * All DRAM tensors partition-major; output bf16, upcast on host.
"""

import sys

sys.path.insert(0, "/opt/trn_rl_repo")

import numpy as np
import ml_dtypes

import concourse.bass as bass
import concourse.tile as tile
from concourse import bacc, mybir
from concourse.bass_utils import run_bass_kernel_spmd

B, S, F = 64, 512, 256
N_CORES = 8
BPC = B // N_CORES  # batches per core
P = 128
KT1 = S // P  # 4 k-tiles over t (x1/Wq contraction)
KT2 = F // P  # 2 k-tiles over f (x2/Wk contraction)
NT = S // P  # 4 m-tiles over u (stage A) / t (stage C)
EPS = 1e-12

F32 = mybir.dt.float32
F32R = mybir.dt.float32r
BF16 = mybir.dt.bfloat16
AF = mybir.ActivationFunctionType
ALU = mybir.AluOpType

BFNP = ml_dtypes.bfloat16

last_results = None  # test harness introspection


def build_nc(reps=1, bpc=BPC):
    nc = bacc.Bacc(
        "TRN2", target_bir_lowering=False, debug=False, num_devices=N_CORES
    )
    # Partition-major packed tensors: [.., P, ktiles, S].
    x1t = nc.declare_dram_parameter("x1t", [bpc, P, KT1, S], F32R, isOutput=False)
    x2t = nc.declare_dram_parameter("x2t", [bpc, P, KT2, S], BF16, isOutput=False)
    wq = nc.declare_dram_parameter("wq", [P, KT1, S], F32R, isOutput=False)
    wqb = nc.declare_dram_parameter("wqb", [P, KT1, S], BF16, isOutput=False)
    x1b0 = nc.declare_dram_parameter("x1b0", [P, KT1, S], BF16, isOutput=False)
    wk = nc.declare_dram_parameter("wk", [P, KT2, S], BF16, isOutput=False)
    wv = nc.declare_dram_parameter("wv", [P, NT, S], BF16, isOutput=False)
    out = nc.declare_dram_parameter("out", [bpc, P, NT, S], BF16, isOutput=True)

    batches = [bb for _ in range(reps) for bb in range(bpc)]

    with tile.TileContext(nc) as tc:
        with (
            tc.tile_pool(name="singles", bufs=1) as singles,
            tc.tile_pool(name="xin", bufs=1) as xin,
            tc.tile_pool(name="work", bufs=2) as work,
            tc.tile_pool(name="small", bufs=2) as small,
            tc.tile_pool(name="outp", bufs=2) as outp,
            tc.tile_pool(name="psA", bufs=2, space="PSUM") as psA,
            tc.tile_pool(name="psY", bufs=3, space="PSUM") as psY,
            tc.tile_pool(name="psR", bufs=1, space="PSUM") as psR,
        ):
            # Startup: x2+wk (0.5MB) land first so batch 0's x2-products can
            # start the PE early; x1/wq (2MB) stream in behind them.
            b0 = batches[0]
            x2_first = xin.tile([P, KT2, S], BF16, tag="x2", bufs=3)
            nc.sync.dma_start(out=x2_first, in_=x2t.ap()[b0])
            wk_t = singles.tile([P, KT2, S], BF16, tag="wk")
            nc.gpsimd.dma_start(out=wk_t, in_=wk.ap())
            # batch 0 runs stage A from a 1.5MB all-bf16 operand set so the
            # PE is fed ~2x sooner; the f32r x1 (epilogue) + wq stream behind.
            wqb_t = singles.tile([P, KT1, S], BF16, tag="wqb")
            nc.scalar.dma_start(out=wqb_t, in_=wqb.ap())
            x1b0_t = xin.tile([P, KT1, S], BF16, tag="x1b0", bufs=1)
            nc.sync.dma_start(out=x1b0_t, in_=x1b0.ap())
            x1_first = xin.tile([P, KT1, S], F32R, tag="x1", bufs=3)
            wq_t = singles.tile([P, KT1, S], F32R, tag="wq")
            nc.scalar.dma_start(out=wq_t[:, 0:2, :], in_=wq.ap()[:, 0:2, :])
            nc.sync.dma_start(out=x1_first[:, 0:2, :], in_=x1t.ap()[b0, :, 0:2, :])
            nc.scalar.dma_start(out=wq_t[:, 2:4, :], in_=wq.ap()[:, 2:4, :])
            nc.sync.dma_start(out=x1_first[:, 2:4, :], in_=x1t.ap()[b0, :, 2:4, :])
            wv_t = singles.tile([P, NT, S], BF16, tag="wv")
            nc.gpsimd.dma_start(out=wv_t, in_=wv.ap())

            ones_blk = singles.tile([P, P], BF16)
            nc.vector.memset(ones_blk, 1.0)
            warm_sb = singles.tile([P, S], BF16, tag="warm")
            nc.vector.memset(warm_sb, 0.0)
            eps_t = singles.tile([P, 1], F32)
            nc.vector.memset(eps_t, EPS)

            def pe_warm(n):
                """Dummy matmuls that keep the TensorE p-state high while it
                would otherwise idle (results never read).  The target cycles
                the single psR bank, ordered before the next rowsum by WAW."""
                warm_ps = psR.tile([P, S], F32, tag="rowsum")
                for _ in range(n):
                    nc.tensor.matmul(
                        warm_ps, ones_blk, warm_sb, start=True, stop=True
                    )

            # Ramp the PE during the startup DMA window.
            pe_warm(9)

            def stage_a(b, x1_sb, x2_sb, mid_cb=None, x2_lead=False, wq_sb=None):
                """scores matmuls in u-tile pairs sharing one 2-bank PSUM
                tile, tanh+exp over pairs.  mid_cb (if set) is emitted
                between the two pair-halves so the previous batch's rowsum
                overlaps this batch's remaining matmuls.  x2_lead emits the
                x2@Wk products first (batch 0: only 0.5MB of operands gate
                the PE while x1/wq are still in flight)."""
                expz = work.tile([P, NT, S], BF16, tag="expz", bufs=3)
                for half in range(NT // 2):
                    sc = psA.tile([P, 2, S], F32, tag="scores")
                    for j in range(2):
                        ut = half * 2 + j
                        us = slice(ut * P, (ut + 1) * P)
                        wq_use = wq_sb if wq_sb is not None else wq_t
                        prods = [
                            (wq_use[:, kt, us], x1_sb[:, kt, :]) for kt in range(KT1)
                        ] + [(wk_t[:, kt, us], x2_sb[:, kt, :]) for kt in range(KT2)]
                        if x2_lead:
                            prods = prods[KT1:] + prods[:KT1]
                        for pi, (l_ap, r_ap) in enumerate(prods):
                            nc.tensor.matmul(
                                sc[:, j, :],
                                l_ap,
                                r_ap,
                                start=(pi == 0),
                                stop=(pi == len(prods) - 1),
                            )
                    tanh_t = work.tile([P, 2, S], F32, tag="tanh")
                    nc.scalar.activation(out=tanh_t, in_=sc, func=AF.Tanh)
                    nc.scalar.activation(
                        out=expz[:, half * 2 : half * 2 + 2, :],
                        in_=tanh_t,
                        func=AF.Exp,
                    )
                    if half == 0 and mid_cb is not None:
                        mid_cb()
                return expz

            def stage_b(b, expz):
                """softmax denominator: the ones-BLOCK rowsum matmul writes
                the denominator replicated across all 128 partitions, so the
                reciprocal lands directly in broadcast form -- no GpSimd
                partition_broadcast needed."""
                rs = psR.tile([P, S], F32, tag="rowsum")
                for ut in range(NT):
                    nc.tensor.matmul(
                        rs,
                        ones_blk,
                        expz[:, ut, :],
                        start=(ut == 0),
                        stop=(ut == NT - 1),
                    )
                rbc = small.tile([P, S], F32, tag="rbc")
                nc.vector.reciprocal_approx_fast(out=rbc, in_=rs)
                return rbc

            def stage_c(b, x1_sb, expz, rbc, drain=False):
                """Y matmuls on raw expz; epilogue q=y*x1 -> w=q*rbc (f32 on
                DVE); sum-of-squares split between ACT Square+accum and DVE
                stt to balance engine load."""
                w_sb = outp.tile([P, NT, S], F32, tag="w", bufs=3)
                sumsq = small.tile([P, NT], F32, tag="sumsq", bufs=4)
                for tt in range(NT):
                    y = psY.tile([P, S], F32, tag="y")
                    for ut in range(NT):
                        nc.tensor.matmul(
                            y,
                            wv_t[:, ut, tt * P : (tt + 1) * P],
                            expz[:, ut, :],
                            start=(ut == 0),
                            stop=(ut == NT - 1),
                        )
                    q_t = small.tile([P, S], F32, tag="q")
                    w_t = w_sb[:, tt, :]
                    nc.vector.tensor_tensor(
                        out=q_t, in0=y, in1=x1_sb[:, tt, :].bitcast(F32), op=ALU.mult
                    )
                    nc.vector.tensor_tensor(out=w_t, in0=q_t, in1=rbc, op=ALU.mult)
                    if tt >= 2 and not drain:
                        scr = small.tile([P, S], F32, tag="scr")
                        nc.vector.scalar_tensor_tensor(
                            out=scr,
                            in0=w_t,
                            scalar=1.0,
                            in1=w_t,
                            op0=ALU.mult,
                            op1=ALU.mult,
                            accum_out=sumsq[:, tt : tt + 1],
                        )
                    else:
                        scr = small.tile([P, S], BF16, tag="scrb")
                        nc.scalar.activation(
                            out=scr,
                            in_=w_t,
                            func=AF.Square,
                            accum_out=sumsq[:, tt : tt + 1],
                        )
                return w_sb, sumsq

            def stage_fin(b, w_sb, sumsq):
                """sqrt (ACT, emitted adjacently for pairs of batches to halve
                activation-table swaps), GpSimd normalize, store bf16."""
                rsq = small.tile([P, NT], F32, tag="rsq", bufs=4)
                nc.scalar.activation(out=rsq, in_=sumsq, func=AF.Sqrt, bias=eps_t)
                ob = outp.tile([P, NT, S], BF16, tag="ob")
                for tt in range(NT):
                    nc.gpsimd.normalize_recip(
                        out_ap=ob[:, tt, :],
                        in_ap=w_sb[:, tt, :],
                        denom_ap=rsq[:, tt : tt + 1],
                    )
                nc.scalar.dma_start(out=out.ap()[b], in_=ob)

            def stage_fin_last(b, w_sb, sumsq):
                """Drain finalize: normalizes split Pool/DVE, two half-DMAs."""
                rsq = small.tile([P, NT], F32, tag="rsq", bufs=4)
                nc.scalar.activation(out=rsq, in_=sumsq, func=AF.Sqrt, bias=eps_t)
                vv = small.tile([P, NT], F32, tag="vv")
                nc.vector.reciprocal_approx_fast(out=vv, in_=rsq)
                ob = outp.tile([P, NT, S], BF16, tag="ob")
                for tt in range(NT):
                    if tt % 2 == 0:
                        nc.gpsimd.normalize_recip(
                            out_ap=ob[:, tt, :],
                            in_ap=w_sb[:, tt, :],
                            denom_ap=rsq[:, tt : tt + 1],
                        )
                    else:
                        nc.vector.tensor_scalar_mul(
                            ob[:, tt, :], w_sb[:, tt, :], vv[:, tt : tt + 1]
                        )
                    if tt == 1:
                        nc.scalar.dma_start(
                            out=out.ap()[b, :, 0:2, :], in_=ob[:, 0:2, :]
                        )
                nc.scalar.dma_start(out=out.ap()[b, :, 2:4, :], in_=ob[:, 2:4, :])

            def dma_x(b):
                t1 = xin.tile([P, KT1, S], F32R, tag="x1", bufs=3)
                nc.sync.dma_start(out=t1[:, 0:2, :], in_=x1t.ap()[b, :, 0:2, :])
                nc.sync.dma_start(out=t1[:, 2:4, :], in_=x1t.ap()[b, :, 2:4, :])
                t2 = xin.tile([P, KT2, S], BF16, tag="x2", bufs=3)
                nc.sync.dma_start(out=t2, in_=x2t.ap()[b])
                return t1, t2

            pending = None  # (b, x1_sb, expz) awaiting stages B+C
            fins = []  # (b, w_sb, sumsq) awaiting finalize, flushed in pairs
            x1_cur, x2_cur = x1_first, x2_first
            for i, b in enumerate(batches):
                if i + 1 < len(batches):
                    nxt = dma_x(batches[i + 1])
                else:
                    nxt = (None, None)
                prev = pending
                hold = {}

                def mid_cb():
                    hold["rbc"] = stage_b(prev[0], prev[2])

                expz = stage_a(
                    b,
                    x1b0_t if i == 0 else x1_cur,
                    x2_cur,
                    mid_cb if prev is not None else None,
                    x2_lead=(i == 0),
                    wq_sb=wqb_t if i == 0 else None,
                )
                if prev is not None:
                    fins.append(
                        (prev[0],) + stage_c(prev[0], prev[1], prev[2], hold["rbc"])
                    )
                    if len(fins) == 2:
                        for f in fins:
                            stage_fin(*f)
                        fins = []
                pending = (b, x1_cur, expz)
                x1_cur, x2_cur = nxt
            # drain: dummy matmuls keep the PE clock hot while the last
            # batch's exp completes.  The pending finalize flushes FIRST so
            # its Pool normalizes overlap the last batch's stage-C matmuls;
            # the drain's ACT squares and sqrt then share one sqrt-table
            # epoch (Square lives in both activation-table sets).
            pe_warm(9)
            rbc_last = stage_b(pending[0], pending[2])
            for f in fins:
                stage_fin(*f)
            last_c = stage_c(
                pending[0], pending[1], pending[2], rbc_last, drain=True
            )
            stage_fin_last(pending[0], *last_c)

    nc.compile()
    return nc


def _pack_pmajor(a, nchunks):
    """[.., nchunks*P, S] -> [.., P, nchunks, S] partition-major contiguous."""
    lead = a.shape[:-2]
    a = a.reshape(lead + (nchunks, P, S))
    perm = tuple(range(len(lead))) + (len(lead) + 1, len(lead), len(lead) + 2)
    return np.ascontiguousarray(a.transpose(perm))


_nc_cache = None


def kernel(x1, x2, W_query, W_key, W_value, _trace=False):
    global _nc_cache, last_results
    x1t = _pack_pmajor(
        np.asarray(x1, dtype=np.float32).transpose(0, 2, 1), KT1
    )  # [B, P, KT1, S]
    x2t = _pack_pmajor(
        np.asarray(x2, dtype=np.float32).transpose(0, 2, 1).astype(BFNP), KT2
    )
    wq = _pack_pmajor(np.asarray(W_query, dtype=np.float32), KT1)
    wqb = _pack_pmajor(np.asarray(W_query, dtype=np.float32).astype(BFNP), KT1)
    x1b_all = _pack_pmajor(
        np.asarray(x1, dtype=np.float32).transpose(0, 2, 1).astype(BFNP), KT1
    )
    wk = _pack_pmajor(np.asarray(W_key, dtype=np.float32).astype(BFNP), KT2)
    wv = _pack_pmajor(np.asarray(W_value, dtype=np.float32).astype(BFNP), NT)

    if _nc_cache is None:
        _nc_cache = build_nc()
    nc = _nc_cache

    in_maps = []
    for c in range(N_CORES):
        sl = slice(c * BPC, (c + 1) * BPC)
        in_maps.append(
            {
                "x1t": x1t[sl],
                "x2t": x2t[sl],
                "wq": wq,
                "wqb": wqb,
                "x1b0": x1b_all[c * BPC],
                "wk": wk,
                "wv": wv,
            }
        )
    res = run_bass_kernel_spmd(
        nc, in_maps, core_ids=list(range(N_CORES)), trace=_trace
    )
    last_results = res
    # out: [bpc, P, NT, S] bf16 -> outT [B, S, S] -> untranspose
    outs = [np.asarray(res.results[c]["out"]) for c in range(N_CORES)]
    outT = np.concatenate(outs, axis=0).astype(np.float32)
    outT = outT.transpose(0, 2, 1, 3).reshape(B, S, S)
    return np.ascontiguousarray(outT.transpose(0, 2, 1))
